# revision 21
# baseline (speedup 1.0000x reference)
"""BERT-with-RoPE attention layer on 8 Trainium2 NeuronCores.

Sharding: core c handles (batch b = c//2, sequence-half hf = c%2).
Each core computes k/v for its batch's full 2048 rows (k/v work duplicated
2x across the pair — cheap) and q + attention + out-projection for its own
1024 query rows, so the 8 output shards are disjoint and the host gather is
a pure concatenation (no collectives).

Matmuls run in bf16 (weights/activations rounded on host or at eviction;
fp32 PSUM accumulate; softmax math in fp32) — the vLLM-standard precision
for this layer. The RoPE halves-swap runs as a float32r PE permutation
matmul so rope arithmetic stays fp32 until the final bf16 rounding.

Device dataflow per core:
  phase A: qT/kT = Wqkv^T @ xT in [outcol, seq] layout (N=1024 bf16 mms),
           bias via ACT eviction, NeoX RoPE as raw*cos + Pswap^T@(raw*sin).
  phase B: v = xT^T @ Wv in natural [seq, dcol] layout, SBUF-resident,
           with a ones column appended per head for softmax sums.
  phase C: per head-pair: scoresT[s2,s1] for both heads as row-tiled
           concurrent matmuls, exp on ACT (scale=1/8 fused, FD=1024),
           ctx^T + sums accumulated in PSUM via the ones column; normalize
           with reciprocal + gpsimd partition-broadcast.
  phase D: outT[Hout,s1] = Wout^T-slices @ ctxT, bias via ACT eviction.
"""

import os
import numpy as np

B, S, H = 4, 2048, 1024
NH, DH = 16, 64
HALF = DH // 2
SQ = S // 2  # query rows per core
KC = H // 128  # hidden contraction chunks
ROPE_BASE = 10000.0
N_CORES = 8

_nc_cache = None
last_results = None


def _build_nc():
    import concourse.bacc as bacc
    import concourse.mybir as mybir
    from concourse.tile import TileContext

    f32 = mybir.dt.float32
    f32r = mybir.dt.float32r
    bf16 = mybir.dt.bfloat16
    Exp = mybir.ActivationFunctionType.Exp
    Ident = mybir.ActivationFunctionType.Identity
    Copy = mybir.ActivationFunctionType.Copy
    MUL = mybir.AluOpType.mult
    ADD = mybir.AluOpType.add

    nc = bacc.Bacc(None, target_bir_lowering=False)

    xT_d = nc.dram_tensor("xT", [KC, 128, S], bf16, kind="ExternalInput")
    wqk_d = nc.dram_tensor("wqk", [16, KC, 128, 128], bf16, kind="ExternalInput")
    wv_d = nc.dram_tensor("wv", [KC, 128, H], bf16, kind="ExternalInput")
    wout_d = nc.dram_tensor("wout", [8, KC, 128, 128], bf16, kind="ExternalInput")
    pswap_d = nc.dram_tensor("pswap", [128, 128], f32r, kind="ExternalInput")
    cosk_d = nc.dram_tensor("cosk", [128, S], f32, kind="ExternalInput")
    sink_d = nc.dram_tensor("sink", [128, S], f32, kind="ExternalInput")
    bqk_d = nc.dram_tensor("bqk", [128, 16], f32, kind="ExternalInput")
    ones_d = nc.dram_tensor("ones", [128, 16], bf16, kind="ExternalInput")
    boutp_d = nc.dram_tensor("boutp", [128, 8], f32, kind="ExternalInput")
    out_d = nc.dram_tensor("outT", [8, 128, SQ], f32, kind="ExternalOutput")
    debug = bool(int(os.environ.get("KERNEL_DEBUG", "0") or "0"))
    if debug:
        dq_d = nc.dram_tensor("dq", [128, KC, SQ], bf16, kind="ExternalOutput")
        dk_d = nc.dram_tensor("dk", [128, KC, S], bf16, kind="ExternalOutput")
        dv_d = nc.dram_tensor("dv", [128, 16, NH, DH + 1], bf16, kind="ExternalOutput")
        dctx_d = nc.dram_tensor("dctx", [128, KC, SQ], bf16, kind="ExternalOutput")

    with TileContext(nc) as tc:
        with (
            tc.tile_pool(name="const", bufs=1) as const,
            tc.tile_pool(name="persist", bufs=1) as persist,
        ):
            pswap_sb = const.tile([128, 128], f32r)
            nc.sync.dma_start(pswap_sb[:, :], pswap_d[:, :])
            bqk_sb = const.tile([128, 16], f32)
            nc.sync.dma_start(bqk_sb[:, :], bqk_d[:, :])
            boutp_sb = const.tile([128, 8], f32)
            nc.sync.dma_start(boutp_sb[:, :], boutp_d[:, :])

            qTr = persist.tile([128, 8, SQ], bf16)
            kTr = persist.tile([128, 8, S], bf16)
            # v resident in SBUF: [s2_in_blk, s2_blk, head, dcol+ones]
            v_sb = persist.tile([128, 16, NH, DH + 1], bf16)

            # ---------------- phase A: q/k projection + rope -------------
            with tc.tile_pool(name="xTp", bufs=1) as xTp:
                xT_sb = xTp.tile([128, KC, S], bf16)
                for c in range(KC):
                    nc.sync.dma_start(xT_sb[:, c, :], xT_d[c, :, :])

                with (
                    tc.tile_pool(name="mapp", bufs=1) as mapp,
                    tc.tile_pool(name="wqkp", bufs=2) as wqkp,
                    tc.tile_pool(name="ropep", bufs=4) as ropep,
                    tc.tile_pool(name="psQK", bufs=2, space="PSUM") as psQK,
                    tc.tile_pool(name="psSW", bufs=2, space="PSUM") as psSW,
                ):
                    cosk_sb = mapp.tile([128, S], f32)
                    nc.sync.dma_start(cosk_sb[:, :], cosk_d[:, :])
                    sink_sb = mapp.tile([128, S], f32)
                    nc.sync.dma_start(sink_sb[:, :], sink_d[:, :])

                    pending = []

                    def _stage2(state):
                        raw, tt, sl, oc = state
                        sw = psSW.tile([128, SQ], f32, tag="sw", name="sw")
                        for hv in range(2):
                            nc.tensor.matmul(
                                sw[:, hv * 512 : (hv + 1) * 512],
                                pswap_sb[:, :], tt[:, hv * 512 : (hv + 1) * 512],
                                start=True, stop=True,
                            )
                        cc = ropep.tile([128, SQ], f32, tag="cc", name="cc")
                        nc.gpsimd.tensor_tensor(
                            cc[:, :], raw[:, :], cosk_sb[:, sl], MUL
                        )
                        if oc < 8:
                            dst = qTr[:, oc, :]
                        else:
                            dst = kTr[:, oc - 8, sl]
                        nc.vector.tensor_tensor(dst, cc[:, :], sw[:, :], ADD)

                    for oc in range(16):
                        wts = []
                        for c in range(KC):
                            wt = wqkp.tile([128, 128], bf16, tag=f"w{c}", name=f"wt{c}")
                            nc.sync.dma_start(wt[:, :], wqk_d[oc, c, :, :])
                            wts.append(wt)
                        nspan = 1 if oc < 8 else 2
                        for sp in range(nspan):
                            sl = slice(sp * SQ, (sp + 1) * SQ)
                            ps = psQK.tile([128, SQ], f32, tag="psQK", name="psQK_t")
                            for c in range(KC):
                                for hv in range(2):
                                    nc.tensor.matmul(
                                        ps[:, hv * 512 : (hv + 1) * 512],
                                        wts[c][:, :],
                                        xT_sb[:, c, sp * SQ + hv * 512 : sp * SQ + (hv + 1) * 512],
                                        start=(c == 0), stop=(c == KC - 1),
                                    )
                            raw = ropep.tile([128, SQ], f32, tag="raw", name="raw")
                            nc.scalar.activation(
                                raw[:, :], ps[:, :], Ident, bias=bqk_sb[:, oc : oc + 1]
                            )
                            tt = ropep.tile([128, SQ], f32r, tag="tt", name="tt")
                            nc.vector.tensor_tensor(
                                tt[:, :], raw[:, :], sink_sb[:, sl], MUL
                            )
                            pending.append((raw, tt, sl, oc))
                            if len(pending) > 1:
                                _stage2(pending.pop(0))
                    while pending:
                        _stage2(pending.pop(0))

                # ------------- phase B: v projection (SBUF-resident) -----
                with (
                    tc.tile_pool(name="wvp", bufs=1) as wvp,
                    tc.tile_pool(name="psV", bufs=3, space="PSUM") as psV,
                ):
                    wvts = []
                    for c in range(KC):
                        wvt = wvp.tile([128, H], bf16, tag=f"wv{c}", name=f"wvt{c}")
                        nc.sync.dma_start(wvt[:, :], wv_d[c, :, :])
                        wvts.append(wvt)
                    for sb in range(16):
                        ps = psV.tile([128, H], f32, tag="psV", name="psV_t")
                        for c in range(KC):
                            for hv in range(2):
                                nc.tensor.matmul(
                                    ps[:, hv * 512 : (hv + 1) * 512],
                                    xT_sb[:, c, sb * 128 : (sb + 1) * 128],
                                    wvts[c][:, hv * 512 : (hv + 1) * 512],
                                    start=(c == 0), stop=(c == KC - 1),
                                )
                        nc.scalar.activation(
                            v_sb[:, sb, :, 0:DH],
                            ps.rearrange("p (h d) -> p h d", h=NH),
                            Copy,
                        )
                        nc.sync.dma_start(v_sb[:, sb, :, DH : DH + 1], ones_d[:, :])

            # ---------------- phase C: attention (head pairs) ------------
            ctxp = tc.alloc_tile_pool(name="ctxp", bufs=1)
            ctxT = ctxp.tile([128, KC, SQ], bf16)
            with (
                tc.tile_pool(name="expp", bufs=4) as expp,
                tc.tile_pool(name="scrp", bufs=2) as scrp,
                tc.tile_pool(name="psSc", bufs=3, space="PSUM") as psSc,
                tc.tile_pool(name="psCtx", bufs=1, space="PSUM") as psCtx,
            ):
                for pr in range(8):
                    for hf in range(2):
                        s1 = slice(hf * 512, (hf + 1) * 512)
                        cE = psCtx.tile([128, 512], f32, tag="ctxe", name="cE")
                        cO = psCtx.tile([128, 512], f32, tag="ctxo", name="cO")
                        for blk in range(16):
                            sc = psSc.tile([128, 2, 512], f32, tag="sc", name="sc")
                            for par in range(2):
                                rs = par * 64
                                nc.tensor.matmul(
                                    sc[:, par, :],
                                    kTr[rs : rs + 64, pr, blk * 128 : (blk + 1) * 128],
                                    qTr[rs : rs + 64, pr, s1],
                                    start=True, stop=True,
                                )
                            et = expp.tile([128, 2, 512], bf16, tag="et", name="et")
                            nc.scalar.activation(
                                et[:, :, :], sc[:, :, :], Exp, scale=0.125
                            )
                            st, sp_ = (blk == 0), (blk == 15)
                            nc.tensor.matmul(
                                cE[0 : DH + 1, :],
                                v_sb[:, blk, 2 * pr, :], et[:, 0, :],
                                start=st, stop=sp_,
                            )
                            nc.tensor.matmul(
                                cO[0 : DH + 1, :],
                                v_sb[:, blk, 2 * pr + 1, :], et[:, 1, :],
                                start=st, stop=sp_,
                            )
                        # epilogue per (pair, s1-half): normalize ctx rows
                        # 0..63 by sums row 64. reciprocal_approx_fast /
                        # partition_broadcast act on tensor partition 0
                        # regardless of AP base, so route the sums row
                        # through partition 0 via DMA.
                        for par, ct in ((0, cE), (1, cO)):
                            scr = scrp.tile([128, 512], f32, tag="scr", name="scr")
                            nc.vector.tensor_copy(scr[64:65, :], ct[64:65, :])
                            scr2 = scrp.tile([1, 512], f32, tag="scr2", name="scr2")
                            nc.sync.dma_start(scr2[0:1, :], scr[64:65, :])
                            bcs = scrp.tile([128, 512], f32, tag="bcs", name="bcs")
                            nc.gpsimd.partition_broadcast(bcs[0:64, :], scr2[0:1, :])
                            bc = scrp.tile([128, 512], f32, tag="bc", name="bc")
                            nc.vector.reciprocal_approx_fast(bc[0:64, :], bcs[0:64, :])
                            if par == 0:
                                nc.vector.tensor_tensor(
                                    ctxT[0:64, pr, s1], ct[0:64, :], bc[0:64, :], MUL
                                )
                            else:
                                tmp = scrp.tile([64, 512], bf16, tag="tmp", name="tmp")
                                nc.vector.tensor_tensor(
                                    tmp[:, :], ct[0:64, :], bc[0:64, :], MUL
                                )
                                nc.sync.dma_start(ctxT[64:128, pr, s1], tmp[:, :])

            if debug:
                nc.sync.dma_start(dq_d[:, :, :], qTr[:, :, :])
                nc.sync.dma_start(dk_d[:, :, :], kTr[:, :, :])
                nc.sync.dma_start(dv_d[:, :, :, :], v_sb[:, :, :, :])
                nc.sync.dma_start(dctx_d[:, :, :], ctxT[:, :, :])

            # ---------------- phase D: out projection -------------------
            with (
                tc.tile_pool(name="woutp", bufs=2) as woutp,
                tc.tile_pool(name="obp", bufs=2) as obp,
                tc.tile_pool(name="psO", bufs=2, space="PSUM") as psO,
            ):
                for hb in range(8):
                    owts = []
                    for c in range(KC):
                        owt = woutp.tile([128, 128], bf16, tag=f"o{c}", name=f"owt{c}")
                        nc.sync.dma_start(owt[:, :], wout_d[hb, c, :, :])
                        owts.append(owt)
                    ps = psO.tile([128, SQ], f32, tag="psO", name="psO_t")
                    for c in range(KC):
                        for hv in range(2):
                            nc.tensor.matmul(
                                ps[:, hv * 512 : (hv + 1) * 512],
                                owts[c][:, :],
                                ctxT[:, c, hv * 512 : (hv + 1) * 512],
                                start=(c == 0), stop=(c == KC - 1),
                            )
                    ob = obp.tile([128, SQ], f32, tag="ob", name="ob")
                    nc.scalar.activation(
                        ob[:, :], ps[:, :], Ident, bias=boutp_sb[:, hb : hb + 1]
                    )
                    nc.sync.dma_start(out_d[hb, :, :], ob[:, :])
            ctxp.release()

    nc.finalize()
    return nc


def _host_prep(positions, hidden_states, Wqkv, bqkv, Wout, bout):
    import ml_dtypes

    bf16 = ml_dtypes.bfloat16
    positions = np.asarray(positions)
    hidden_states = np.asarray(hidden_states, dtype=np.float32)
    Wqkv = np.asarray(Wqkv, dtype=np.float32)
    bqkv = np.asarray(bqkv, dtype=np.float32)
    Wout = np.asarray(Wout, dtype=np.float32)
    bout = np.asarray(bout, dtype=np.float32)

    wqk = np.ascontiguousarray(
        Wqkv[:, : 2 * H].reshape(KC, 128, 16, 128).transpose(2, 0, 1, 3)
    ).astype(bf16)
    wv = np.ascontiguousarray(Wqkv[:, 2 * H :].reshape(KC, 128, H)).astype(bf16)
    wout_t = np.ascontiguousarray(
        Wout.reshape(KC, 128, 8, 128).transpose(2, 0, 1, 3)
    ).astype(bf16)
    bqk = np.ascontiguousarray(bqkv[: 2 * H].reshape(16, 128).T)
    boutp_full = bout.astype(np.float64) + bqkv[2 * H :].astype(
        np.float64
    ) @ Wout.astype(np.float64)
    boutp = np.ascontiguousarray(boutp_full.astype(np.float32).reshape(8, 128).T)

    pswap = np.zeros((128, 128), dtype=np.float32)
    for m in range(128):
        if m % 64 < HALF:
            pswap[m + HALF, m] = -1.0
        else:
            pswap[m - HALF, m] = 1.0

    inv_freq = 1.0 / (ROPE_BASE ** (np.arange(HALF, dtype=np.float64) / HALF))
    rowmap = np.arange(128) % HALF

    in_maps = []
    for c in range(N_CORES):
        b, hf = c // 2, c % 2
        perm = np.concatenate(
            [np.arange(hf * SQ, (hf + 1) * SQ), np.arange((1 - hf) * SQ, (2 - hf) * SQ)]
        )
        x_perm = hidden_states[b][perm]
        xT = np.ascontiguousarray(x_perm.T).reshape(KC, 128, S).astype(bf16)
        pos = positions[perm].astype(np.float64)
        freqs = pos[:, None] * inv_freq[None, :]  # [S, HALF]
        cosk = np.ascontiguousarray(np.cos(freqs).astype(np.float32)[:, rowmap].T)
        sink = np.ascontiguousarray(np.sin(freqs).astype(np.float32)[:, rowmap].T)
        in_maps.append(
            {
                "xT": xT, "wqk": wqk, "wv": wv, "wout": wout_t,
                "pswap": pswap, "cosk": cosk, "sink": sink,
                "bqk": bqk, "boutp": boutp,
                "ones": np.ones((128, 16), dtype=bf16),
            }
        )
    return in_maps


def kernel(positions, hidden_states, Wqkv, bqkv, Wout, bout):
    global _nc_cache, last_results
    from concourse import bass_utils

    if _nc_cache is None:
        _nc_cache = _build_nc()
    nc = _nc_cache

    in_maps = _host_prep(positions, hidden_states, Wqkv, bqkv, Wout, bout)
    res = bass_utils.run_bass_kernel_spmd(
        nc, in_maps, core_ids=list(range(N_CORES)),
        trace=bool(int(os.environ.get("KERNEL_TRACE", "0") or "0")),
    )
    last_results = res

    out = np.empty((B, S, H), dtype=np.float32)
    for c in range(N_CORES):
        b, hf = c // 2, c % 2
        outT = np.asarray(res.results[c]["outT"]).reshape(H, SQ)
        out[b, hf * SQ : (hf + 1) * SQ, :] = outT.T
    return out


# revision 22
# speedup vs baseline: 1.0432x; 1.0432x over previous
"""BERT-with-RoPE attention layer on 8 Trainium2 NeuronCores.

Sharding: core c handles (batch b = c//2, sequence-half hf = c%2).
Each core computes k/v for its batch's full 2048 rows (k/v work duplicated
2x across the pair — cheap) and q + attention + out-projection for its own
1024 query rows, so the 8 output shards are disjoint and the host gather is
a pure concatenation (no collectives).

Matmuls run in bf16 (weights/activations rounded on host or at eviction;
fp32 PSUM accumulate; softmax math in fp32) — the vLLM-standard precision
for this layer. The RoPE halves-swap runs as a float32r PE permutation
matmul so rope arithmetic stays fp32 until the final bf16 rounding.

Device dataflow per core:
  phase A: qT/kT = Wqkv^T @ xT in [outcol, seq] layout (N=1024 bf16 mms),
           bias via ACT eviction, NeoX RoPE as raw*cos + Pswap^T@(raw*sin).
  phase B: v = xT^T @ Wv in natural [seq, dcol] layout, SBUF-resident,
           with a ones column appended per head for softmax sums.
  phase C: per head-pair: scoresT[s2,s1] for both heads as row-tiled
           concurrent matmuls, exp on ACT (scale=1/8 fused, FD=1024),
           ctx^T + sums accumulated in PSUM via the ones column; normalize
           with reciprocal + gpsimd partition-broadcast.
  phase D: outT[Hout,s1] = Wout^T-slices @ ctxT, bias via ACT eviction.
"""

import os
import numpy as np

B, S, H = 4, 2048, 1024
NH, DH = 16, 64
HALF = DH // 2
SQ = S // 2  # query rows per core
KC = H // 128  # hidden contraction chunks
ROPE_BASE = 10000.0
N_CORES = 8

_nc_cache = None
last_results = None


def _build_nc():
    import concourse.bacc as bacc
    import concourse.mybir as mybir
    from concourse.tile import TileContext

    f32 = mybir.dt.float32
    f32r = mybir.dt.float32r
    bf16 = mybir.dt.bfloat16
    Exp = mybir.ActivationFunctionType.Exp
    Ident = mybir.ActivationFunctionType.Identity
    Copy = mybir.ActivationFunctionType.Copy
    MUL = mybir.AluOpType.mult
    ADD = mybir.AluOpType.add

    nc = bacc.Bacc(None, target_bir_lowering=False)

    xT_d = nc.dram_tensor("xT", [KC, 128, S], bf16, kind="ExternalInput")
    wqk_d = nc.dram_tensor("wqk", [16, KC, 128, 128], bf16, kind="ExternalInput")
    wv_d = nc.dram_tensor("wv", [KC, 128, H], bf16, kind="ExternalInput")
    wout_d = nc.dram_tensor("wout", [8, KC, 128, 128], bf16, kind="ExternalInput")
    pswap_d = nc.dram_tensor("pswap", [128, 128], f32r, kind="ExternalInput")
    cosk_d = nc.dram_tensor("cosk", [128, S], f32, kind="ExternalInput")
    sink_d = nc.dram_tensor("sink", [128, S], f32, kind="ExternalInput")
    bqk_d = nc.dram_tensor("bqk", [128, 16], f32, kind="ExternalInput")
    ones_d = nc.dram_tensor("ones", [128, 16], bf16, kind="ExternalInput")
    boutp_d = nc.dram_tensor("boutp", [128, 8], f32, kind="ExternalInput")
    out_d = nc.dram_tensor("outT", [8, 128, SQ], f32, kind="ExternalOutput")
    debug = bool(int(os.environ.get("KERNEL_DEBUG", "0") or "0"))
    if debug:
        dq_d = nc.dram_tensor("dq", [128, KC, SQ], bf16, kind="ExternalOutput")
        dk_d = nc.dram_tensor("dk", [128, KC, S], bf16, kind="ExternalOutput")
        dv_d = nc.dram_tensor("dv", [128, 16, NH, DH + 1], bf16, kind="ExternalOutput")
        dctx_d = nc.dram_tensor("dctx", [128, KC, SQ], bf16, kind="ExternalOutput")

    with TileContext(nc) as tc:
        with (
            tc.tile_pool(name="const", bufs=1) as const,
            tc.tile_pool(name="persist", bufs=1) as persist,
        ):
            pswap_sb = const.tile([128, 128], f32r)
            nc.sync.dma_start(pswap_sb[:, :], pswap_d[:, :])
            bqk_sb = const.tile([128, 16], f32)
            nc.sync.dma_start(bqk_sb[:, :], bqk_d[:, :])
            boutp_sb = const.tile([128, 8], f32)
            nc.sync.dma_start(boutp_sb[:, :], boutp_d[:, :])

            qTr = persist.tile([128, 8, SQ], bf16)
            kTr = persist.tile([128, 8, S], bf16)
            # v resident in SBUF: [s2_in_blk, s2_blk, head, dcol+ones]
            v_sb = persist.tile([128, 16, NH, DH + 1], bf16)

            # ---------------- phase A: q/k projection + rope -------------
            with tc.tile_pool(name="xTp", bufs=1) as xTp:
                xT_sb = xTp.tile([128, KC, S], bf16)
                for c in range(KC):
                    nc.sync.dma_start(xT_sb[:, c, :], xT_d[c, :, :])

                with (
                    tc.tile_pool(name="mapp", bufs=1) as mapp,
                    tc.tile_pool(name="wqkp", bufs=3) as wqkp,
                    tc.tile_pool(name="ropep", bufs=4) as ropep,
                    tc.tile_pool(name="psQK", bufs=2, space="PSUM") as psQK,
                    tc.tile_pool(name="psSW", bufs=2, space="PSUM") as psSW,
                ):
                    cosk_sb = mapp.tile([128, S], f32)
                    nc.sync.dma_start(cosk_sb[:, :], cosk_d[:, :])
                    sink_sb = mapp.tile([128, S], f32)
                    nc.sync.dma_start(sink_sb[:, :], sink_d[:, :])

                    pending = []

                    def _stage2(state):
                        raw, tt, sl, oc = state
                        sw = psSW.tile([128, SQ], f32, tag="sw", name="sw")
                        for hv in range(2):
                            nc.tensor.matmul(
                                sw[:, hv * 512 : (hv + 1) * 512],
                                pswap_sb[:, :], tt[:, hv * 512 : (hv + 1) * 512],
                                start=True, stop=True,
                            )
                        cc = ropep.tile([128, SQ], f32, tag="cc", name="cc")
                        nc.gpsimd.tensor_tensor(
                            cc[:, :], raw[:, :], cosk_sb[:, sl], MUL
                        )
                        if oc < 8:
                            dst = qTr[:, oc, :]
                        else:
                            dst = kTr[:, oc - 8, sl]
                        nc.vector.tensor_tensor(dst, cc[:, :], sw[:, :], ADD)

                    for oc in range(16):
                        wts = []
                        for c in range(KC):
                            wt = wqkp.tile([128, 128], bf16, tag=f"w{c}", name=f"wt{c}")
                            nc.sync.dma_start(wt[:, :], wqk_d[oc, c, :, :])
                            wts.append(wt)
                        nspan = 1 if oc < 8 else 2
                        for sp in range(nspan):
                            sl = slice(sp * SQ, (sp + 1) * SQ)
                            ps = psQK.tile([128, SQ], f32, tag="psQK", name="psQK_t")
                            for c in range(KC):
                                for hv in range(2):
                                    nc.tensor.matmul(
                                        ps[:, hv * 512 : (hv + 1) * 512],
                                        wts[c][:, :],
                                        xT_sb[:, c, sp * SQ + hv * 512 : sp * SQ + (hv + 1) * 512],
                                        start=(c == 0), stop=(c == KC - 1),
                                    )
                            raw = ropep.tile([128, SQ], f32, tag="raw", name="raw")
                            nc.scalar.activation(
                                raw[:, :], ps[:, :], Ident, bias=bqk_sb[:, oc : oc + 1]
                            )
                            tt = ropep.tile([128, SQ], f32r, tag="tt", name="tt")
                            nc.vector.tensor_tensor(
                                tt[:, :], raw[:, :], sink_sb[:, sl], MUL
                            )
                            pending.append((raw, tt, sl, oc))
                            if len(pending) > 1:
                                _stage2(pending.pop(0))
                    while pending:
                        _stage2(pending.pop(0))

                # ------------- phase B: v projection (SBUF-resident) -----
                with (
                    tc.tile_pool(name="wvp", bufs=1) as wvp,
                    tc.tile_pool(name="psV", bufs=3, space="PSUM") as psV,
                ):
                    wvts = []
                    for c in range(KC):
                        wvt = wvp.tile([128, H], bf16, tag=f"wv{c}", name=f"wvt{c}")
                        nc.sync.dma_start(wvt[:, :], wv_d[c, :, :])
                        wvts.append(wvt)
                    for sb in range(16):
                        ps = psV.tile([128, H], f32, tag="psV", name="psV_t")
                        for c in range(KC):
                            for hv in range(2):
                                nc.tensor.matmul(
                                    ps[:, hv * 512 : (hv + 1) * 512],
                                    xT_sb[:, c, sb * 128 : (sb + 1) * 128],
                                    wvts[c][:, hv * 512 : (hv + 1) * 512],
                                    start=(c == 0), stop=(c == KC - 1),
                                )
                        nc.scalar.activation(
                            v_sb[:, sb, :, 0:DH],
                            ps.rearrange("p (h d) -> p h d", h=NH),
                            Copy,
                        )
                        nc.sync.dma_start(v_sb[:, sb, :, DH : DH + 1], ones_d[:, :])

            # ---------------- phase C: attention (head pairs) ------------
            ctxp = tc.alloc_tile_pool(name="ctxp", bufs=1)
            ctxT = ctxp.tile([128, KC, SQ], bf16)
            with (
                tc.tile_pool(name="expp", bufs=6) as expp,
                tc.tile_pool(name="scrp", bufs=3) as scrp,
                tc.tile_pool(name="psSc", bufs=3, space="PSUM") as psSc,
                tc.tile_pool(name="psCtx", bufs=1, space="PSUM") as psCtx,
            ):
                for pr in range(8):
                    for hf in range(2):
                        s1 = slice(hf * 512, (hf + 1) * 512)
                        cE = psCtx.tile([128, 512], f32, tag="ctxe", name="cE")
                        cO = psCtx.tile([128, 512], f32, tag="ctxo", name="cO")
                        for blk in range(16):
                            sc = psSc.tile([128, 2, 512], f32, tag="sc", name="sc")
                            for par in range(2):
                                rs = par * 64
                                nc.tensor.matmul(
                                    sc[:, par, :],
                                    kTr[rs : rs + 64, pr, blk * 128 : (blk + 1) * 128],
                                    qTr[rs : rs + 64, pr, s1],
                                    start=True, stop=True,
                                )
                            et = expp.tile([128, 2, 512], bf16, tag="et", name="et")
                            nc.scalar.activation(
                                et[:, :, :], sc[:, :, :], Exp, scale=0.125
                            )
                            st, sp_ = (blk == 0), (blk == 15)
                            nc.tensor.matmul(
                                cE[0 : DH + 1, :],
                                v_sb[:, blk, 2 * pr, :], et[:, 0, :],
                                start=st, stop=sp_,
                            )
                            nc.tensor.matmul(
                                cO[0 : DH + 1, :],
                                v_sb[:, blk, 2 * pr + 1, :], et[:, 1, :],
                                start=st, stop=sp_,
                            )
                        # epilogue per (pair, s1-half): normalize ctx rows
                        # 0..63 by sums row 64. reciprocal_approx_fast /
                        # partition_broadcast act on tensor partition 0
                        # regardless of AP base, so route the sums row
                        # through partition 0 via DMA.
                        for par, ct in ((0, cE), (1, cO)):
                            scr = scrp.tile([128, 512], f32, tag="scr", name="scr")
                            nc.vector.tensor_copy(scr[64:65, :], ct[64:65, :])
                            scr2 = scrp.tile([1, 512], f32, tag="scr2", name="scr2")
                            nc.sync.dma_start(scr2[0:1, :], scr[64:65, :])
                            bcs = scrp.tile([128, 512], f32, tag="bcs", name="bcs")
                            nc.gpsimd.partition_broadcast(bcs[0:64, :], scr2[0:1, :])
                            bc = scrp.tile([128, 512], f32, tag="bc", name="bc")
                            nc.vector.reciprocal_approx_fast(bc[0:64, :], bcs[0:64, :])
                            if par == 0:
                                nc.vector.tensor_tensor(
                                    ctxT[0:64, pr, s1], ct[0:64, :], bc[0:64, :], MUL
                                )
                            else:
                                tmp = scrp.tile([64, 512], bf16, tag="tmp", name="tmp")
                                nc.vector.tensor_tensor(
                                    tmp[:, :], ct[0:64, :], bc[0:64, :], MUL
                                )
                                nc.sync.dma_start(ctxT[64:128, pr, s1], tmp[:, :])

            if debug:
                nc.sync.dma_start(dq_d[:, :, :], qTr[:, :, :])
                nc.sync.dma_start(dk_d[:, :, :], kTr[:, :, :])
                nc.sync.dma_start(dv_d[:, :, :, :], v_sb[:, :, :, :])
                nc.sync.dma_start(dctx_d[:, :, :], ctxT[:, :, :])

            # ---------------- phase D: out projection -------------------
            with (
                tc.tile_pool(name="woutp", bufs=3) as woutp,
                tc.tile_pool(name="obp", bufs=2) as obp,
                tc.tile_pool(name="psO", bufs=2, space="PSUM") as psO,
            ):
                for hb in range(8):
                    owts = []
                    for c in range(KC):
                        owt = woutp.tile([128, 128], bf16, tag=f"o{c}", name=f"owt{c}")
                        nc.sync.dma_start(owt[:, :], wout_d[hb, c, :, :])
                        owts.append(owt)
                    ps = psO.tile([128, SQ], f32, tag="psO", name="psO_t")
                    for c in range(KC):
                        for hv in range(2):
                            nc.tensor.matmul(
                                ps[:, hv * 512 : (hv + 1) * 512],
                                owts[c][:, :],
                                ctxT[:, c, hv * 512 : (hv + 1) * 512],
                                start=(c == 0), stop=(c == KC - 1),
                            )
                    ob = obp.tile([128, SQ], f32, tag="ob", name="ob")
                    nc.scalar.activation(
                        ob[:, :], ps[:, :], Ident, bias=boutp_sb[:, hb : hb + 1]
                    )
                    nc.sync.dma_start(out_d[hb, :, :], ob[:, :])
            ctxp.release()

    nc.finalize()
    return nc


def _host_prep(positions, hidden_states, Wqkv, bqkv, Wout, bout):
    import ml_dtypes

    bf16 = ml_dtypes.bfloat16
    positions = np.asarray(positions)
    hidden_states = np.asarray(hidden_states, dtype=np.float32)
    Wqkv = np.asarray(Wqkv, dtype=np.float32)
    bqkv = np.asarray(bqkv, dtype=np.float32)
    Wout = np.asarray(Wout, dtype=np.float32)
    bout = np.asarray(bout, dtype=np.float32)

    wqk = np.ascontiguousarray(
        Wqkv[:, : 2 * H].reshape(KC, 128, 16, 128).transpose(2, 0, 1, 3)
    ).astype(bf16)
    wv = np.ascontiguousarray(Wqkv[:, 2 * H :].reshape(KC, 128, H)).astype(bf16)
    wout_t = np.ascontiguousarray(
        Wout.reshape(KC, 128, 8, 128).transpose(2, 0, 1, 3)
    ).astype(bf16)
    bqk = np.ascontiguousarray(bqkv[: 2 * H].reshape(16, 128).T)
    boutp_full = bout.astype(np.float64) + bqkv[2 * H :].astype(
        np.float64
    ) @ Wout.astype(np.float64)
    boutp = np.ascontiguousarray(boutp_full.astype(np.float32).reshape(8, 128).T)

    pswap = np.zeros((128, 128), dtype=np.float32)
    for m in range(128):
        if m % 64 < HALF:
            pswap[m + HALF, m] = -1.0
        else:
            pswap[m - HALF, m] = 1.0

    inv_freq = 1.0 / (ROPE_BASE ** (np.arange(HALF, dtype=np.float64) / HALF))
    rowmap = np.arange(128) % HALF

    in_maps = []
    for c in range(N_CORES):
        b, hf = c // 2, c % 2
        perm = np.concatenate(
            [np.arange(hf * SQ, (hf + 1) * SQ), np.arange((1 - hf) * SQ, (2 - hf) * SQ)]
        )
        x_perm = hidden_states[b][perm]
        xT = np.ascontiguousarray(x_perm.T).reshape(KC, 128, S).astype(bf16)
        pos = positions[perm].astype(np.float64)
        freqs = pos[:, None] * inv_freq[None, :]  # [S, HALF]
        cosk = np.ascontiguousarray(np.cos(freqs).astype(np.float32)[:, rowmap].T)
        sink = np.ascontiguousarray(np.sin(freqs).astype(np.float32)[:, rowmap].T)
        in_maps.append(
            {
                "xT": xT, "wqk": wqk, "wv": wv, "wout": wout_t,
                "pswap": pswap, "cosk": cosk, "sink": sink,
                "bqk": bqk, "boutp": boutp,
                "ones": np.ones((128, 16), dtype=bf16),
            }
        )
    return in_maps


def kernel(positions, hidden_states, Wqkv, bqkv, Wout, bout):
    global _nc_cache, last_results
    from concourse import bass_utils

    if _nc_cache is None:
        _nc_cache = _build_nc()
    nc = _nc_cache

    in_maps = _host_prep(positions, hidden_states, Wqkv, bqkv, Wout, bout)
    res = bass_utils.run_bass_kernel_spmd(
        nc, in_maps, core_ids=list(range(N_CORES)),
        trace=bool(int(os.environ.get("KERNEL_TRACE", "0") or "0")),
    )
    last_results = res

    out = np.empty((B, S, H), dtype=np.float32)
    for c in range(N_CORES):
        b, hf = c // 2, c % 2
        outT = np.asarray(res.results[c]["outT"]).reshape(H, SQ)
        out[b, hf * SQ : (hf + 1) * SQ, :] = outT.T
    return out


# revision 23
# speedup vs baseline: 1.0433x; 1.0001x over previous
"""BERT-with-RoPE attention layer on 8 Trainium2 NeuronCores.

Sharding: core c handles (batch b = c//2, sequence-half hf = c%2).
Each core computes k/v for its batch's full 2048 rows (k/v work duplicated
2x across the pair — cheap) and q + attention + out-projection for its own
1024 query rows, so the 8 output shards are disjoint and the host gather is
a pure concatenation (no collectives).

Matmuls run in bf16 (weights/activations rounded on host or at eviction;
fp32 PSUM accumulate; softmax math in fp32) — the vLLM-standard precision
for this layer. The RoPE halves-swap runs as a float32r PE permutation
matmul so rope arithmetic stays fp32 until the final bf16 rounding.

Device dataflow per core:
  phase A: qT/kT = Wqkv^T @ xT in [outcol, seq] layout (N=1024 bf16 mms),
           bias via ACT eviction, NeoX RoPE as raw*cos + Pswap^T@(raw*sin).
  phase B: v = xT^T @ Wv in natural [seq, dcol] layout, SBUF-resident,
           with a ones column appended per head for softmax sums.
  phase C: per head-pair: scoresT[s2,s1] for both heads as row-tiled
           concurrent matmuls, exp on ACT (scale=1/8 fused, FD=1024),
           ctx^T + sums accumulated in PSUM via the ones column; normalize
           with reciprocal + gpsimd partition-broadcast.
  phase D: outT[Hout,s1] = Wout^T-slices @ ctxT, bias via ACT eviction.
"""

import os
import numpy as np

B, S, H = 4, 2048, 1024
NH, DH = 16, 64
HALF = DH // 2
SQ = S // 2  # query rows per core
KC = H // 128  # hidden contraction chunks
ROPE_BASE = 10000.0
N_CORES = 8

_nc_cache = None
last_results = None


def _build_nc():
    import concourse.bacc as bacc
    import concourse.mybir as mybir
    from concourse.tile import TileContext

    f32 = mybir.dt.float32
    f32r = mybir.dt.float32r
    bf16 = mybir.dt.bfloat16
    Exp = mybir.ActivationFunctionType.Exp
    Ident = mybir.ActivationFunctionType.Identity
    Copy = mybir.ActivationFunctionType.Copy
    MUL = mybir.AluOpType.mult
    ADD = mybir.AluOpType.add

    nc = bacc.Bacc(None, target_bir_lowering=False)

    xT_d = nc.dram_tensor("xT", [KC, 128, S], bf16, kind="ExternalInput")
    wqk_d = nc.dram_tensor("wqk", [16, KC, 128, 128], bf16, kind="ExternalInput")
    wv_d = nc.dram_tensor("wv", [KC, 128, H], bf16, kind="ExternalInput")
    wout_d = nc.dram_tensor("wout", [8, KC, 128, 128], bf16, kind="ExternalInput")
    pswap_d = nc.dram_tensor("pswap", [128, 128], f32r, kind="ExternalInput")
    cosk_d = nc.dram_tensor("cosk", [128, S], f32, kind="ExternalInput")
    sink_d = nc.dram_tensor("sink", [128, S], f32, kind="ExternalInput")
    bqk_d = nc.dram_tensor("bqk", [128, 16], f32, kind="ExternalInput")
    ones_d = nc.dram_tensor("ones", [128, 16], bf16, kind="ExternalInput")
    boutp_d = nc.dram_tensor("boutp", [128, 8], f32, kind="ExternalInput")
    out_d = nc.dram_tensor("outT", [8, 128, SQ], f32, kind="ExternalOutput")
    debug = bool(int(os.environ.get("KERNEL_DEBUG", "0") or "0"))
    if debug:
        dq_d = nc.dram_tensor("dq", [128, KC, SQ], bf16, kind="ExternalOutput")
        dk_d = nc.dram_tensor("dk", [128, KC, S], bf16, kind="ExternalOutput")
        dv_d = nc.dram_tensor("dv", [128, 16, NH, DH + 1], bf16, kind="ExternalOutput")
        dctx_d = nc.dram_tensor("dctx", [128, KC, SQ], bf16, kind="ExternalOutput")

    with TileContext(nc) as tc:
        with (
            tc.tile_pool(name="const", bufs=1) as const,
            tc.tile_pool(name="persist", bufs=1) as persist,
        ):
            pswap_sb = const.tile([128, 128], f32r)
            nc.sync.dma_start(pswap_sb[:, :], pswap_d[:, :])
            bqk_sb = const.tile([128, 16], f32)
            nc.sync.dma_start(bqk_sb[:, :], bqk_d[:, :])
            boutp_sb = const.tile([128, 8], f32)
            nc.sync.dma_start(boutp_sb[:, :], boutp_d[:, :])

            qTr = persist.tile([128, 8, SQ], bf16)
            kTr = persist.tile([128, 8, S], bf16)
            # v resident in SBUF: [s2_in_blk, s2_blk, head, dcol+ones]
            v_sb = persist.tile([128, 16, NH, DH + 1], bf16)

            # ---------------- phase A: q/k projection + rope -------------
            with tc.tile_pool(name="xTp", bufs=1) as xTp:
                xT_sb = xTp.tile([128, KC, S], bf16)
                for c in range(KC):
                    nc.sync.dma_start(xT_sb[:, c, :], xT_d[c, :, :])

                with (
                    tc.tile_pool(name="mapp", bufs=1) as mapp,
                    tc.tile_pool(name="wqkp", bufs=3) as wqkp,
                    tc.tile_pool(name="ropep", bufs=5) as ropep,
                    tc.tile_pool(name="psQK", bufs=2, space="PSUM") as psQK,
                    tc.tile_pool(name="psSW", bufs=2, space="PSUM") as psSW,
                ):
                    cosk_sb = mapp.tile([128, S], f32)
                    nc.sync.dma_start(cosk_sb[:, :], cosk_d[:, :])
                    sink_sb = mapp.tile([128, S], f32)
                    nc.sync.dma_start(sink_sb[:, :], sink_d[:, :])

                    pending = []

                    def _stage2(state):
                        raw, tt, sl, oc = state
                        sw = psSW.tile([128, SQ], f32, tag="sw", name="sw")
                        for hv in range(2):
                            nc.tensor.matmul(
                                sw[:, hv * 512 : (hv + 1) * 512],
                                pswap_sb[:, :], tt[:, hv * 512 : (hv + 1) * 512],
                                start=True, stop=True,
                            )
                        cc = ropep.tile([128, SQ], f32, tag="cc", name="cc")
                        nc.gpsimd.tensor_tensor(
                            cc[:, :], raw[:, :], cosk_sb[:, sl], MUL
                        )
                        if oc < 8:
                            dst = qTr[:, oc, :]
                        else:
                            dst = kTr[:, oc - 8, sl]
                        nc.vector.tensor_tensor(dst, cc[:, :], sw[:, :], ADD)

                    for oc in range(16):
                        wts = []
                        for c in range(KC):
                            wt = wqkp.tile([128, 128], bf16, tag=f"w{c}", name=f"wt{c}")
                            nc.sync.dma_start(wt[:, :], wqk_d[oc, c, :, :])
                            wts.append(wt)
                        nspan = 1 if oc < 8 else 2
                        for sp in range(nspan):
                            sl = slice(sp * SQ, (sp + 1) * SQ)
                            ps = psQK.tile([128, SQ], f32, tag="psQK", name="psQK_t")
                            for c in range(KC):
                                for hv in range(2):
                                    nc.tensor.matmul(
                                        ps[:, hv * 512 : (hv + 1) * 512],
                                        wts[c][:, :],
                                        xT_sb[:, c, sp * SQ + hv * 512 : sp * SQ + (hv + 1) * 512],
                                        start=(c == 0), stop=(c == KC - 1),
                                    )
                            raw = ropep.tile([128, SQ], f32, tag="raw", name="raw")
                            nc.scalar.activation(
                                raw[:, :], ps[:, :], Ident, bias=bqk_sb[:, oc : oc + 1]
                            )
                            tt = ropep.tile([128, SQ], f32r, tag="tt", name="tt")
                            nc.vector.tensor_tensor(
                                tt[:, :], raw[:, :], sink_sb[:, sl], MUL
                            )
                            pending.append((raw, tt, sl, oc))
                            if len(pending) > 1:
                                _stage2(pending.pop(0))
                    while pending:
                        _stage2(pending.pop(0))

                # ------------- phase B: v projection (SBUF-resident) -----
                with (
                    tc.tile_pool(name="wvp", bufs=1) as wvp,
                    tc.tile_pool(name="psV", bufs=3, space="PSUM") as psV,
                ):
                    wvts = []
                    for c in range(KC):
                        wvt = wvp.tile([128, H], bf16, tag=f"wv{c}", name=f"wvt{c}")
                        nc.sync.dma_start(wvt[:, :], wv_d[c, :, :])
                        wvts.append(wvt)
                    for sb in range(16):
                        ps = psV.tile([128, H], f32, tag="psV", name="psV_t")
                        for c in range(KC):
                            for hv in range(2):
                                nc.tensor.matmul(
                                    ps[:, hv * 512 : (hv + 1) * 512],
                                    xT_sb[:, c, sb * 128 : (sb + 1) * 128],
                                    wvts[c][:, hv * 512 : (hv + 1) * 512],
                                    start=(c == 0), stop=(c == KC - 1),
                                )
                        nc.scalar.activation(
                            v_sb[:, sb, :, 0:DH],
                            ps.rearrange("p (h d) -> p h d", h=NH),
                            Copy,
                        )
                        nc.sync.dma_start(v_sb[:, sb, :, DH : DH + 1], ones_d[:, :])

            # ---------------- phase C: attention (head pairs) ------------
            ctxp = tc.alloc_tile_pool(name="ctxp", bufs=1)
            ctxT = ctxp.tile([128, KC, SQ], bf16)
            with (
                tc.tile_pool(name="expp", bufs=6) as expp,
                tc.tile_pool(name="scrp", bufs=4) as scrp,
                tc.tile_pool(name="psSc", bufs=3, space="PSUM") as psSc,
                tc.tile_pool(name="psCtx", bufs=1, space="PSUM") as psCtx,
            ):
                for pr in range(8):
                    for hf in range(2):
                        s1 = slice(hf * 512, (hf + 1) * 512)
                        cE = psCtx.tile([128, 512], f32, tag="ctxe", name="cE")
                        cO = psCtx.tile([128, 512], f32, tag="ctxo", name="cO")
                        for blk in range(16):
                            sc = psSc.tile([128, 2, 512], f32, tag="sc", name="sc")
                            for par in range(2):
                                rs = par * 64
                                nc.tensor.matmul(
                                    sc[:, par, :],
                                    kTr[rs : rs + 64, pr, blk * 128 : (blk + 1) * 128],
                                    qTr[rs : rs + 64, pr, s1],
                                    start=True, stop=True,
                                )
                            et = expp.tile([128, 2, 512], bf16, tag="et", name="et")
                            nc.scalar.activation(
                                et[:, :, :], sc[:, :, :], Exp, scale=0.125
                            )
                            st, sp_ = (blk == 0), (blk == 15)
                            nc.tensor.matmul(
                                cE[0 : DH + 1, :],
                                v_sb[:, blk, 2 * pr, :], et[:, 0, :],
                                start=st, stop=sp_,
                            )
                            nc.tensor.matmul(
                                cO[0 : DH + 1, :],
                                v_sb[:, blk, 2 * pr + 1, :], et[:, 1, :],
                                start=st, stop=sp_,
                            )
                        # epilogue per (pair, s1-half): normalize ctx rows
                        # 0..63 by sums row 64. reciprocal_approx_fast /
                        # partition_broadcast act on tensor partition 0
                        # regardless of AP base, so route the sums row
                        # through partition 0 via DMA.
                        for par, ct in ((0, cE), (1, cO)):
                            scr = scrp.tile([128, 512], f32, tag="scr", name="scr")
                            nc.vector.tensor_copy(scr[64:65, :], ct[64:65, :])
                            scr2 = scrp.tile([1, 512], f32, tag="scr2", name="scr2")
                            nc.sync.dma_start(scr2[0:1, :], scr[64:65, :])
                            bcs = scrp.tile([128, 512], f32, tag="bcs", name="bcs")
                            nc.gpsimd.partition_broadcast(bcs[0:64, :], scr2[0:1, :])
                            bc = scrp.tile([128, 512], f32, tag="bc", name="bc")
                            nc.vector.reciprocal_approx_fast(bc[0:64, :], bcs[0:64, :])
                            if par == 0:
                                nc.vector.tensor_tensor(
                                    ctxT[0:64, pr, s1], ct[0:64, :], bc[0:64, :], MUL
                                )
                            else:
                                tmp = scrp.tile([64, 512], bf16, tag="tmp", name="tmp")
                                nc.vector.tensor_tensor(
                                    tmp[:, :], ct[0:64, :], bc[0:64, :], MUL
                                )
                                nc.sync.dma_start(ctxT[64:128, pr, s1], tmp[:, :])

            if debug:
                nc.sync.dma_start(dq_d[:, :, :], qTr[:, :, :])
                nc.sync.dma_start(dk_d[:, :, :], kTr[:, :, :])
                nc.sync.dma_start(dv_d[:, :, :, :], v_sb[:, :, :, :])
                nc.sync.dma_start(dctx_d[:, :, :], ctxT[:, :, :])

            # ---------------- phase D: out projection -------------------
            with (
                tc.tile_pool(name="woutp", bufs=3) as woutp,
                tc.tile_pool(name="obp", bufs=3) as obp,
                tc.tile_pool(name="psO", bufs=2, space="PSUM") as psO,
            ):
                for hb in range(8):
                    owts = []
                    for c in range(KC):
                        owt = woutp.tile([128, 128], bf16, tag=f"o{c}", name=f"owt{c}")
                        nc.sync.dma_start(owt[:, :], wout_d[hb, c, :, :])
                        owts.append(owt)
                    ps = psO.tile([128, SQ], f32, tag="psO", name="psO_t")
                    for c in range(KC):
                        for hv in range(2):
                            nc.tensor.matmul(
                                ps[:, hv * 512 : (hv + 1) * 512],
                                owts[c][:, :],
                                ctxT[:, c, hv * 512 : (hv + 1) * 512],
                                start=(c == 0), stop=(c == KC - 1),
                            )
                    ob = obp.tile([128, SQ], f32, tag="ob", name="ob")
                    nc.scalar.activation(
                        ob[:, :], ps[:, :], Ident, bias=boutp_sb[:, hb : hb + 1]
                    )
                    nc.sync.dma_start(out_d[hb, :, :], ob[:, :])
            ctxp.release()

    nc.finalize()
    return nc


def _host_prep(positions, hidden_states, Wqkv, bqkv, Wout, bout):
    import ml_dtypes

    bf16 = ml_dtypes.bfloat16
    positions = np.asarray(positions)
    hidden_states = np.asarray(hidden_states, dtype=np.float32)
    Wqkv = np.asarray(Wqkv, dtype=np.float32)
    bqkv = np.asarray(bqkv, dtype=np.float32)
    Wout = np.asarray(Wout, dtype=np.float32)
    bout = np.asarray(bout, dtype=np.float32)

    wqk = np.ascontiguousarray(
        Wqkv[:, : 2 * H].reshape(KC, 128, 16, 128).transpose(2, 0, 1, 3)
    ).astype(bf16)
    wv = np.ascontiguousarray(Wqkv[:, 2 * H :].reshape(KC, 128, H)).astype(bf16)
    wout_t = np.ascontiguousarray(
        Wout.reshape(KC, 128, 8, 128).transpose(2, 0, 1, 3)
    ).astype(bf16)
    bqk = np.ascontiguousarray(bqkv[: 2 * H].reshape(16, 128).T)
    boutp_full = bout.astype(np.float64) + bqkv[2 * H :].astype(
        np.float64
    ) @ Wout.astype(np.float64)
    boutp = np.ascontiguousarray(boutp_full.astype(np.float32).reshape(8, 128).T)

    pswap = np.zeros((128, 128), dtype=np.float32)
    for m in range(128):
        if m % 64 < HALF:
            pswap[m + HALF, m] = -1.0
        else:
            pswap[m - HALF, m] = 1.0

    inv_freq = 1.0 / (ROPE_BASE ** (np.arange(HALF, dtype=np.float64) / HALF))
    rowmap = np.arange(128) % HALF

    in_maps = []
    for c in range(N_CORES):
        b, hf = c // 2, c % 2
        perm = np.concatenate(
            [np.arange(hf * SQ, (hf + 1) * SQ), np.arange((1 - hf) * SQ, (2 - hf) * SQ)]
        )
        x_perm = hidden_states[b][perm]
        xT = np.ascontiguousarray(x_perm.T).reshape(KC, 128, S).astype(bf16)
        pos = positions[perm].astype(np.float64)
        freqs = pos[:, None] * inv_freq[None, :]  # [S, HALF]
        cosk = np.ascontiguousarray(np.cos(freqs).astype(np.float32)[:, rowmap].T)
        sink = np.ascontiguousarray(np.sin(freqs).astype(np.float32)[:, rowmap].T)
        in_maps.append(
            {
                "xT": xT, "wqk": wqk, "wv": wv, "wout": wout_t,
                "pswap": pswap, "cosk": cosk, "sink": sink,
                "bqk": bqk, "boutp": boutp,
                "ones": np.ones((128, 16), dtype=bf16),
            }
        )
    return in_maps


def kernel(positions, hidden_states, Wqkv, bqkv, Wout, bout):
    global _nc_cache, last_results
    from concourse import bass_utils

    if _nc_cache is None:
        _nc_cache = _build_nc()
    nc = _nc_cache

    in_maps = _host_prep(positions, hidden_states, Wqkv, bqkv, Wout, bout)
    res = bass_utils.run_bass_kernel_spmd(
        nc, in_maps, core_ids=list(range(N_CORES)),
        trace=bool(int(os.environ.get("KERNEL_TRACE", "0") or "0")),
    )
    last_results = res

    out = np.empty((B, S, H), dtype=np.float32)
    for c in range(N_CORES):
        b, hf = c // 2, c % 2
        outT = np.asarray(res.results[c]["outT"]).reshape(H, SQ)
        out[b, hf * SQ : (hf + 1) * SQ, :] = outT.T
    return out
